# revision 39
# baseline (speedup 1.0000x reference)
"""Causal linear attention (ELU+1 feature map) for Trainium2, 8 NeuronCores.

Sharding: core c handles batch b = c // 4 and head-group g = c % 4
(4 heads of 64 dims -> a 256-feature slice of the QKV/O projections).
Each core computes its partial O-projection output (2048, 1024); the host
sums the 4 partials per batch and adds bo.

v3: fp16 operands, resident inputs, PE tile-mode batching (all 64-row
matmuls grouped, all 128-row grouped, transposes grouped), head-pair
merged transposes / kv-update matmuls, V-bias folded into the vst
eviction add, fp16 phi intermediates.
"""

import numpy as np

import concourse.bacc as bacc
import concourse.bass as bass
import concourse.mybir as mybir
import concourse.tile as tile
from concourse.bass import ds, ts
from concourse.bass_utils import run_bass_kernel_spmd
from concourse.masks import make_identity, make_upper_triangular

B, S, H_DIM = 2, 2048, 1024
N_HEADS, HEAD_DIM = 16, 64
EPS = 1e-6

N_CORES = 8
HPC = 4                  # heads per core
O = HPC * HEAD_DIM       # 256: per-core projection feature slice
CH = 128                 # key chunk
QB = 512                 # query block
N_CH = S // CH           # 16
N_QB = S // QB           # 4
CPB = QB // CH           # 4 chunks per query block
KI = H_DIM // 128        # 8 contraction chunks
SB = 512                 # projection s-block width
N_SB = S // SB           # 4

FP32 = mybir.dt.float32
FP16 = mybir.dt.float16

AF = mybir.ActivationFunctionType


def _emit(tc):
    nc = tc.nc
    xq = nc.dram_tensor("xq", [128, KI, S], FP16, kind="ExternalInput").ap()
    xk = nc.dram_tensor("xk", [128, KI, S], FP16, kind="ExternalInput").ap()
    xv = nc.dram_tensor("xv", [128, KI, S], FP16, kind="ExternalInput").ap()
    wq = nc.dram_tensor("wq", [128, KI, O], FP16, kind="ExternalInput").ap()
    wk = nc.dram_tensor("wk", [128, KI, O], FP16, kind="ExternalInput").ap()
    wv = nc.dram_tensor("wv", [128, KI, O], FP16, kind="ExternalInput").ap()
    wo = nc.dram_tensor("wo", [128, 2, H_DIM], FP16, kind="ExternalInput").ap()
    bqd = nc.dram_tensor("bq", [128, 2], FP32, kind="ExternalInput").ap()
    bkd = nc.dram_tensor("bk", [128, 2], FP32, kind="ExternalInput").ap()
    bvd = nc.dram_tensor("bv", [1, O], FP16, kind="ExternalInput").ap()
    y = nc.dram_tensor("y", [S, H_DIM], FP16, kind="ExternalOutput").ap()

    with tc.tile_pool(name="singles", bufs=1) as singles:
        _emit_body(tc, singles, xq, xk, xv, wq, wk, wv, wo, bqd, bkd, bvd, y)


def _emit_body(tc, singles, xq, xk, xv, wq, wk, wv, wo, bqd, bkd, bvd, y):
    nc = tc.nc
    # --- resident weights / constants -------------------------------------
    wq_s = singles.tile([128, KI, O], FP16, tag="wq")
    wk_s = singles.tile([128, KI, O], FP16, tag="wk")
    wv_s = singles.tile([128, KI, O], FP16, tag="wv")
    wo_s = singles.tile([128, 2, H_DIM], FP16, tag="wo")
    nc.gpsimd.dma_start(wq_s[:], wq)
    nc.gpsimd.dma_start(wv_s[:], wv)
    nc.gpsimd.dma_start(wk_s[:], wk)
    bq_s = singles.tile([128, 2], FP32, tag="bq")
    bk_s = singles.tile([128, 2], FP32, tag="bk")
    bv_s = singles.tile([1, O], FP16, tag="bv")
    nc.gpsimd.dma_start(bq_s[:], bqd)
    nc.gpsimd.dma_start(bk_s[:], bkd)
    nc.gpsimd.dma_start(bv_s[:], bvd)
    bvb = singles.tile([128, O], FP16, tag="bvb")
    nc.gpsimd.partition_broadcast(bvb[:], bv_s[:])

    # --- resident activations / inputs ------------------------------------
    xq_s = singles.tile([128, KI, S], FP16, tag="xq")
    xk_s = singles.tile([128, KI, S], FP16, tag="xk")
    xv_s = singles.tile([128, KI, S], FP16, tag="xv")
    # block 0 is split per contraction chunk so the first matmuls can
    # start ~1us in instead of waiting for a whole-block descriptor burst;
    # xk of each block is consumed last within emit_A, so defer it behind
    # the next block's xv/xq
    scol0 = ds(0, SB)
    scol1 = ds(SB, SB)
    for ic in range(KI):
        nc.sync.dma_start(xv_s[:, ic, scol0], xv[:, ic, scol0])
        nc.sync.dma_start(xq_s[:, ic, scol0], xq[:, ic, scol0])
    nc.sync.dma_start(xk_s[:, :, scol0], xk[:, :, scol0])
    for ic in range(KI):
        nc.sync.dma_start(xv_s[:, ic, scol1], xv[:, ic, scol1])
        nc.sync.dma_start(xq_s[:, ic, scol1], xq[:, ic, scol1])
    nc.sync.dma_start(xk_s[:, :, scol1], xk[:, :, scol1])
    # wo is not needed until the first emit_C (~30us in); transfer it after
    # the first two input blocks so they aren't bandwidth-starved
    nc.sync.dma_start(wo_s[:], wo)
    for sb in range(2, N_SB):
        scol = ds(sb * SB, SB)
        nc.sync.dma_start(xv_s[:, :, scol], xv[:, :, scol])
        nc.sync.dma_start(xq_s[:, :, scol], xq[:, :, scol])
        nc.sync.dma_start(xk_s[:, :, scol], xk[:, :, scol])

    ident = singles.tile([128, 128], FP16, tag="ident")
    make_identity(nc, ident[:])
    # 4-fold tiled causal mask for masking a whole query block in one op
    umask4 = singles.tile([128, CPB, 128], FP16, tag="umask4")
    for cj in range(CPB):
        make_upper_triangular(nc, umask4[:, cj, :], val=1.0, diag=True)

    qpt = [singles.tile([128, S], FP16, tag=f"qpt{m}", name=f"qpt{m}") for m in range(2)]
    kpt = [singles.tile([128, S], FP16, tag=f"kpt{m}", name=f"kpt{m}") for m in range(2)]
    # pair-major vaug: vst[:, c, mt, 0:65] = [v | 1] of even head,
    # vst[:, c, mt, 65:130] = [v | 1] of odd head of pair mt
    vst = singles.tile([128, N_CH, 2, 130], FP16, tag="vst")
    nc.gpsimd.memset(vst[:, :, :, 64:65], 1.0)
    nc.gpsimd.memset(vst[:, :, :, 129:130], 1.0)
    outt = [singles.tile([128, S], FP16, tag=f"outt{c}", name=f"outt{c}") for c in range(2)]
    # pair-merged kv state, quadrant layout matching p_kv:
    # [rows 0:64, cols 0:65] = even head, [rows 64:128, cols 65:130] = odd
    kv2_sb = [
        singles.tile([128, 130], FP32, tag=f"kv2sb{mt}", name=f"kv2sb{mt}")
        for mt in range(2)
    ]
    for mt in range(2):
        nc.gpsimd.memset(kv2_sb[mt][:], 0.0)

    with (
        tc.tile_pool(name="phi", bufs=4) as phi_pool,
        tc.tile_pool(name="ssb", bufs=6) as ssb_pool,
        tc.tile_pool(name="knb", bufs=4) as kn_pool,
        tc.tile_pool(name="den", bufs=2) as den_pool,
        tc.tile_pool(name="kvs", bufs=12) as kvs_pool,
        tc.tile_pool(name="yt", bufs=3) as yt_pool,
    ):

        def phi_evict(p_x, b_x, m, dst, scol, pref):
            e_t = phi_pool.tile([128, SB], FP16, tag="e", name=f"e_{pref}")
            nc.scalar.activation(e_t[:], p_x[:], AF.Exp, bias=b_x[:, ds(m, 1)])
            r_t = phi_pool.tile([128, SB], FP16, tag="r", name=f"r_{pref}")
            nc.scalar.activation(r_t[:], p_x[:], AF.Relu, bias=b_x[:, ds(m, 1)])
            nc.vector.tensor_scalar_min(e_t[:], e_t[:], 1.0)
            nc.vector.tensor_add(dst[:, scol], r_t[:], e_t[:])

        def emit_A(sb, pqk_pool, pv_pool):
            scol = ds(sb * SB, SB)
            p_q = [pqk_pool.tile([128, SB], FP32, tag=f"m{m}", name=f"pq{m}_{sb}")
                   for m in range(2)]

            def qk_half(p_x, w_s, m):
                for ic in range(KI):
                    nc.tensor.matmul(
                        p_x[:], w_s[:, ic, ts(m, 128)], xq_s[:, ic, scol]
                        if w_s is wq_s else xk_s[:, ic, scol],
                        start=(ic == 0), stop=(ic == KI - 1),
                    )

            def v_half(hf):
                p_v = pv_pool.tile([128, SB], FP32, tag="v", name=f"pv{hf}_{sb}")
                for ic in range(KI):
                    for st in (2 * hf, 2 * hf + 1):
                        nc.tensor.matmul(
                            p_v[:, ds((st % 2) * O, O)],
                            xv_s[:, ic, ds(sb * SB + st * 128, 128)],
                            wv_s[:, ic, :],
                            start=(ic == 0 and st % 2 == 0),
                            stop=(ic == KI - 1 and st % 2 == 1),
                        )
                for st in (2 * hf, 2 * hf + 1):
                    c = sb * 4 + st
                    nc.vector.tensor_add(
                        vst[:, c].rearrange(
                            "p mt (two s) -> p mt two s", two=2)[:, :, :, 0:64],
                        p_v[:, ds((st % 2) * O, O)].rearrange(
                            "p (mt two d) -> p mt two d", mt=2, two=2),
                        bvb[:].rearrange("p (mt two d) -> p mt two d", mt=2, two=2),
                    )

            # order: q0, v01, q1, v23, k0, k1 — each PSUM reuse gap is
            # covered by >=1.7us of PE work while the evict chain drains
            qk_half(p_q[0], wq_s, 0)
            v_half(0)
            qk_half(p_q[1], wq_s, 1)
            v_half(1)
            for m in range(2):
                phi_evict(p_q[m], bq_s, m, qpt[m], scol, f"q{m}_{sb}")
            p_k = [pqk_pool.tile([128, SB], FP32, tag=f"m{m}", name=f"pk{m}_{sb}")
                   for m in range(2)]
            qk_half(p_k[0], wk_s, 0)
            qk_half(p_k[1], wk_s, 1)
            for m in range(2):
                phi_evict(p_k[m], bk_s, m, kpt[m], scol, f"k{m}_{sb}")

        snap_tiles = {}

        def kv_update(sb, mt, pbig_pool):
            """State += block-sb outer products for pair mt; runs early,
            hoisted into the A-phases (needs only kpt/vst of block sb)."""
            # -- pair transposes: 4 matmuls into one PSUM bank --------------
            p_t4 = pbig_pool.tile([128, CPB, 128], FP16, tag="big",
                                  name=f"pt4_{sb}_{mt}")
            for cj in range(CPB):
                c = sb * CPB + cj
                nc.tensor.matmul(
                    p_t4[:, cj, :], kpt[mt][:, ds(c * CH, CH)], ident[:],
                    is_transpose=True,
                    start=(cj == 0), stop=(cj == CPB - 1),
                )
            # single merged eviction of all 4 transposed chunks
            kn4 = kn_pool.tile([128, CPB, 128], FP16, tag="kn",
                               name=f"kn4_{sb}_{mt}")
            nc.vector.tensor_copy(kn4[:], p_t4[:])
            # -- pair kv updates: 4 matmuls accumulated in PSUM -------------
            # out rows 0-63 = even head feats, 64-127 = odd head feats;
            # cols 0-64 = even head vaug, 65-129 = odd head vaug.
            # Only the two matching quadrants are used.
            p_kv = pbig_pool.tile([128, 130], FP32, tag="big",
                                  name=f"pkv{sb}_{mt}")
            for cj in range(CPB):
                c = sb * CPB + cj
                nc.tensor.matmul(
                    p_kv[:], kn4[:, cj, :], vst[:, c, mt, :],
                    start=(cj == 0), stop=(cj == CPB - 1),
                )
            # single pair-state add; the two mismatched quadrants carry
            # cross-head garbage that is never read
            nc.vector.tensor_add(kv2_sb[mt][:], kv2_sb[mt][:], p_kv[:])

        def take_snap(qb):
            """Snapshot the state all heads see for query-block qb (taken
            between kv_update(qb-1) and kv_update(qb))."""
            for h in range(HPC):
                mt, prow = h // 2, 64 * (h % 2)
                snap = kvs_pool.tile([128, 65], FP16, tag="snap",
                                     name=f"snap{qb}_{h}")
                nc.vector.tensor_copy(
                    snap[ds(prow, 64), :],
                    kv2_sb[mt][ds(prow, 64), ds(65 * (h % 2), 65)],
                )
                snap_tiles[(qb, h)] = snap

        def attn_pair(qb, pair, pnum_pool, pbig_pool, paux_pool):
            """Attention for a head pair (same mt) of query-block qb.
            PE matmuls batched by tile mode: [scores 64-mode],
            [num (128,128) mode], [inter x2 64-mode]."""
            mt = pair[0] // 2
            ctxs = []
            for h in pair:
                prow = 64 * (h % 2)
                p_num = pnum_pool.tile([65, QB], FP32, tag=f"n{h % 2}",
                                       name=f"num{qb}_{h}")
                ctxs.append((h, prow, qpt[mt][ds(prow, 64), :],
                             kpt[mt][ds(prow, 64), :], p_num))

            # -- scores, split diag / off-diagonal, all (64,128) mode -------
            # p_diag bank: the 4 causal diagonal chunks at cols cj*128;
            # p_off1 bank: cj=0 off (384 wide) at 0:384 + cj=2 off (128) at
            # 384:512; p_off2: cj=1 off (256 wide)
            sd_ts, so1_ts, so2_ts = {}, {}, {}
            for h, prow, qp_h, kp_h, p_num in ctxs:
                qb0 = qb * QB
                p_diag = pbig_pool.tile([128, QB], FP32, tag="big",
                                        name=f"pd{qb}_{h}")
                for cj in range(CPB):
                    c = qb * CPB + cj
                    nc.tensor.matmul(
                        p_diag[:, ts(cj, CH)], kp_h[:, ds(c * CH, CH)],
                        qp_h[:, ds(qb0 + cj * CH, CH)],
                        start=(cj == 0), stop=(cj == CPB - 1),
                    )
                p_off1 = pbig_pool.tile([128, QB], FP32, tag="big",
                                        name=f"po1{qb}_{h}")
                nc.tensor.matmul(
                    p_off1[:, 0:384], kp_h[:, ds((qb * CPB) * CH, CH)],
                    qp_h[:, ds(qb0 + CH, 384)], start=True, stop=False,
                )
                nc.tensor.matmul(
                    p_off1[:, 384:512], kp_h[:, ds((qb * CPB + 2) * CH, CH)],
                    qp_h[:, ds(qb0 + 384, 128)], start=False, stop=True,
                )
                p_off2 = pbig_pool.tile([128, 256], FP32, tag="big",
                                        name=f"po2{qb}_{h}")
                nc.tensor.matmul(
                    p_off2[:], kp_h[:, ds((qb * CPB + 1) * CH, CH)],
                    qp_h[:, ds(qb0 + 2 * CH, 256)], start=True, stop=True,
                )
                # evictions: one masked mul + two plain copies per head
                s_d = ssb_pool.tile([128, QB], FP16, tag="sd", name=f"sd{qb}_{h}")
                nc.vector.tensor_mul(s_d[:], p_diag[:], umask4[:])
                s_o1 = ssb_pool.tile([128, QB], FP16, tag="so1",
                                     name=f"so1{qb}_{h}")
                nc.scalar.copy(s_o1[:], p_off1[:])
                s_o2 = ssb_pool.tile([128, 256], FP16, tag="so2",
                                     name=f"so2{qb}_{h}")
                nc.scalar.copy(s_o2[:], p_off2[:])
                sd_ts[h], so1_ts[h], so2_ts[h] = s_d, s_o1, s_o2
            # -- num: 7 matmuls per head, (128,128) mode --------------------
            for h, prow, qp_h, kp_h, p_num in ctxs:
                def vslc(cj):
                    return vst[:, qb * CPB + cj, mt, ds(65 * (h % 2), 65)]
                nc.tensor.matmul(p_num[:, ts(0, CH)], vslc(0),
                                 sd_ts[h][:, ts(0, CH)], start=True, stop=False)
                nc.tensor.matmul(p_num[:, ds(CH, 384)], vslc(0),
                                 so1_ts[h][:, 0:384], start=False, stop=False)
                nc.tensor.matmul(p_num[:, ts(1, CH)], vslc(1),
                                 sd_ts[h][:, ts(1, CH)], start=False, stop=False)
                nc.tensor.matmul(p_num[:, ds(2 * CH, 256)], vslc(1),
                                 so2_ts[h][:], start=False, stop=False)
                nc.tensor.matmul(p_num[:, ts(2, CH)], vslc(2),
                                 sd_ts[h][:, ts(2, CH)], start=False, stop=False)
                nc.tensor.matmul(p_num[:, ds(3 * CH, CH)], vslc(2),
                                 so1_ts[h][:, 384:512], start=False, stop=False)
                nc.tensor.matmul(p_num[:, ts(3, CH)], vslc(3),
                                 sd_ts[h][:, ts(3, CH)], start=False,
                                 stop=(qb == 0))
            # -- inter: 2 matmuls, (64,128) mode ----------------------------
            if qb > 0:
                for h, prow, qp_h, kp_h, p_num in ctxs:
                    nc.tensor.matmul(
                        p_num[:], snap_tiles[(qb, h)][ds(prow, 64), :],
                        qp_h[:, ds(qb * QB, QB)], start=False, stop=True,
                    )
            # -- den / outt (pair-merged reciprocal chain, free-dim packed) --
            den2 = den_pool.tile([1, 2, QB], FP32, tag="den", name=f"den{qb}_{mt}")
            for h, prow, qp_h, kp_h, p_num in ctxs:
                nc.scalar.copy(den2[:, h % 2, :], p_num[ds(64, 1), :])
            rden2 = den_pool.tile([1, 2, QB], FP32, tag="rden", name=f"rden{qb}_{mt}")
            nc.vector.reciprocal_approx_fast(rden2[:], den2[:])
            rd16 = den_pool.tile([1, 2, QB], FP16, tag="rd16", name=f"rd16{qb}_{mt}")
            nc.vector.tensor_copy(rd16[:], rden2[:])
            for h, prow, qp_h, kp_h, p_num in ctxs:
                bc_t = den_pool.tile([64, QB], FP16, tag="bc", name=f"bc{qb}_{h}")
                nc.gpsimd.partition_broadcast(bc_t[:], rd16[:, h % 2, :])
                nc.vector.tensor_mul(
                    outt[mt][ds(prow, 64), ds(qb * QB, QB)], p_num[0:64, :], bc_t[:]
                )

        def emit_C(qb, pbig_pool):
            for st in range(qb * CPB, (qb + 1) * CPB):
                y_t = yt_pool.tile([128, H_DIM], FP16, tag="y", name=f"yt{st}")
                for n in range(2):
                    p_o = pbig_pool.tile([128, 512], FP32, tag="big",
                                         name=f"po{st}_{n}")
                    for ct in range(2):
                        nc.tensor.matmul(
                            p_o[:], outt[ct][:, ts(st, 128)],
                            wo_s[:, ct, ts(n, 512)],
                            start=(ct == 0), stop=(ct == 1),
                        )
                    if n == 0:
                        nc.scalar.copy(y_t[:, ts(n, 512)], p_o[:])
                    else:
                        nc.vector.tensor_copy(y_t[:, ts(n, 512)], p_o[:])
                nc.gpsimd.dma_start(y[ds(st * 128, 128), :], y_t[:])

        # single PSUM allocation for the whole kernel — no mid-kernel
        # pool-transition barrier: 2 (qk) + 1 (v) + 2 (num) + 3 (flow) = 8
        with (
            tc.tile_pool(name="pqk", bufs=1, space="PSUM") as pqk_pool,
            tc.tile_pool(name="pv", bufs=1, space="PSUM") as pv_pool,
            tc.tile_pool(name="pnum", bufs=1, space="PSUM") as pnum_e,
            tc.tile_pool(name="pbig", bufs=3, space="PSUM") as pbig_e,
        ):
            # warm-up matmuls on the identity while the first input DMAs
            # land: keeps the PE_HAM activity window busy so the clock gate
            # releases to 2.4 GHz before the real work starts
            p_warm = pbig_e.tile([128, 128], FP32, tag="big", name="warm")
            for i in range(16):
                nc.tensor.matmul(p_warm[:], ident[:], ident[:],
                                 start=True, stop=True)
            emit_A(0, pqk_pool, pv_pool)
            emit_A(1, pqk_pool, pv_pool)
            kv_update(0, 0, pbig_e)
            kv_update(0, 1, pbig_e)
            take_snap(1)
            kv_update(1, 0, pbig_e)
            kv_update(1, 1, pbig_e)
            take_snap(2)
            attn_pair(0, [0, 1], pnum_e, pbig_e, None)
            attn_pair(0, [2, 3], pnum_e, pbig_e, None)
            emit_A(2, pqk_pool, pv_pool)
            kv_update(2, 0, pbig_e)
            kv_update(2, 1, pbig_e)
            take_snap(3)
            emit_C(0, pbig_e)
            attn_pair(1, [0, 1], pnum_e, pbig_e, None)
            attn_pair(1, [2, 3], pnum_e, pbig_e, None)
            emit_A(3, pqk_pool, pv_pool)
            emit_C(1, pbig_e)
            attn_pair(2, [0, 1], pnum_e, pbig_e, None)
            attn_pair(2, [2, 3], pnum_e, pbig_e, None)
            attn_pair(3, [0, 1], pnum_e, pbig_e, None)
            emit_C(2, pbig_e)
            attn_pair(3, [2, 3], pnum_e, pbig_e, None)
            emit_C(3, pbig_e)


_PROGRAM = None


def _get_program():
    global _PROGRAM
    if _PROGRAM is None:
        nc = bacc.Bacc("TRN2", target_bir_lowering=False, debug=False)
        with tile.TileContext(nc) as tc:
            _emit(tc)
        nc.compile()
        _PROGRAM = nc
    return _PROGRAM


def kernel(query, key, value, Wq, bq, Wk, bk, Wv, bv, Wo, bo, _trace=False):
    query, key, value = (np.asarray(a, np.float32) for a in (query, key, value))
    Wq, Wk, Wv, Wo = (np.asarray(a, np.float32) for a in (Wq, Wk, Wv, Wo))
    bq, bk, bv, bo = (np.asarray(a, np.float32) for a in (bq, bk, bv, bo))

    def xslice(x):  # (2048, 1024) -> (128, 8, 2048) fp16, contraction-chunked
        return np.ascontiguousarray(
            x.T.reshape(KI, 128, S).transpose(1, 0, 2)
        ).astype(np.float16)

    def wslice(W, g):  # (1024, 256) -> (128, 8, 256) contraction-chunked
        wt = W[g * O:(g + 1) * O].T  # (1024, 256)
        return np.ascontiguousarray(
            wt.reshape(KI, 128, O).transpose(1, 0, 2)
        ).astype(np.float16)

    xmaps = [
        {
            "xq": xslice(query[b]),
            "xk": xslice(key[b]),
            "xv": xslice(value[b]),
        }
        for b in range(B)
    ]
    in_maps = []
    for c in range(N_CORES):
        b, g = divmod(c, 4)
        sl = slice(g * O, (g + 1) * O)
        in_maps.append({
            **xmaps[b],
            "wq": wslice(Wq, g),
            "wk": wslice(Wk, g),
            "wv": wslice(Wv, g),
            "wo": np.ascontiguousarray(
                Wo[:, sl].T.reshape(2, 128, H_DIM).transpose(1, 0, 2)
            ).astype(np.float16),
            "bq": np.ascontiguousarray(bq[sl].reshape(2, 128).T),
            "bk": np.ascontiguousarray(bk[sl].reshape(2, 128).T),
            "bv": np.ascontiguousarray(bv[sl].reshape(1, O)).astype(np.float16),
        })

    nc = _get_program()
    res = run_bass_kernel_spmd(
        nc, in_maps, core_ids=list(range(N_CORES)), trace=_trace
    )
    out = np.empty((B, S, H_DIM), np.float32)
    for b in range(B):
        acc = res.results[4 * b]["y"].astype(np.float64)
        for g in range(1, 4):
            acc += res.results[4 * b + g]["y"]
        out[b] = (acc + bo).astype(np.float32)
    if _trace:
        kernel.last_result = res
    return out


# revision 41
# speedup vs baseline: 1.0472x; 1.0472x over previous
"""Causal linear attention (ELU+1 feature map) for Trainium2, 8 NeuronCores.

Sharding: core c handles batch b = c // 4 and head-group g = c % 4
(4 heads of 64 dims -> a 256-feature slice of the QKV/O projections).
Each core computes its partial O-projection output (2048, 1024); the host
sums the 4 partials per batch and adds bo.

v3: fp16 operands, resident inputs, PE tile-mode batching (all 64-row
matmuls grouped, all 128-row grouped, transposes grouped), head-pair
merged transposes / kv-update matmuls, V-bias folded into the vst
eviction add, fp16 phi intermediates.
"""

import numpy as np

import concourse.bacc as bacc
import concourse.bass as bass
import concourse.mybir as mybir
import concourse.tile as tile
from concourse.bass import ds, ts
from concourse.bass_utils import run_bass_kernel_spmd
from concourse.masks import make_identity, make_upper_triangular

B, S, H_DIM = 2, 2048, 1024
N_HEADS, HEAD_DIM = 16, 64
EPS = 1e-6

N_CORES = 8
HPC = 4                  # heads per core
O = HPC * HEAD_DIM       # 256: per-core projection feature slice
CH = 128                 # key chunk
QB = 512                 # query block
N_CH = S // CH           # 16
N_QB = S // QB           # 4
CPB = QB // CH           # 4 chunks per query block
KI = H_DIM // 128        # 8 contraction chunks
SB = 512                 # projection s-block width
N_SB = S // SB           # 4

FP32 = mybir.dt.float32
FP16 = mybir.dt.float16

AF = mybir.ActivationFunctionType


def _emit(tc):
    nc = tc.nc
    xq = nc.dram_tensor("xq", [128, KI, S], FP16, kind="ExternalInput").ap()
    xk = nc.dram_tensor("xk", [128, KI, S], FP16, kind="ExternalInput").ap()
    xv = nc.dram_tensor("xv", [128, KI, S], FP16, kind="ExternalInput").ap()
    wq = nc.dram_tensor("wq", [128, KI, O], FP16, kind="ExternalInput").ap()
    wk = nc.dram_tensor("wk", [128, KI, O], FP16, kind="ExternalInput").ap()
    wv = nc.dram_tensor("wv", [128, KI, O], FP16, kind="ExternalInput").ap()
    wo = nc.dram_tensor("wo", [128, 2, H_DIM], FP16, kind="ExternalInput").ap()
    bqd = nc.dram_tensor("bq", [128, 2], FP32, kind="ExternalInput").ap()
    bkd = nc.dram_tensor("bk", [128, 2], FP32, kind="ExternalInput").ap()
    bvd = nc.dram_tensor("bv", [1, O], FP16, kind="ExternalInput").ap()
    y = nc.dram_tensor("y", [S, H_DIM], FP16, kind="ExternalOutput").ap()

    with tc.tile_pool(name="singles", bufs=1) as singles:
        _emit_body(tc, singles, xq, xk, xv, wq, wk, wv, wo, bqd, bkd, bvd, y)


def _emit_body(tc, singles, xq, xk, xv, wq, wk, wv, wo, bqd, bkd, bvd, y):
    nc = tc.nc
    # --- resident weights / constants -------------------------------------
    wq_s = singles.tile([128, KI, O], FP16, tag="wq")
    wk_s = singles.tile([128, KI, O], FP16, tag="wk")
    wv_s = singles.tile([128, KI, O], FP16, tag="wv")
    wo_s = singles.tile([128, 2, H_DIM], FP16, tag="wo")
    nc.gpsimd.dma_start(wq_s[:], wq)
    nc.gpsimd.dma_start(wv_s[:], wv)
    nc.gpsimd.dma_start(wk_s[:], wk)
    nc.gpsimd.dma_start(wo_s[:], wo)
    bq_s = singles.tile([128, 2], FP32, tag="bq")
    bk_s = singles.tile([128, 2], FP32, tag="bk")
    bv_s = singles.tile([1, O], FP16, tag="bv")
    nc.gpsimd.dma_start(bq_s[:], bqd)
    nc.gpsimd.dma_start(bk_s[:], bkd)
    nc.gpsimd.dma_start(bv_s[:], bvd)
    bvb = singles.tile([128, O], FP16, tag="bvb")
    nc.gpsimd.partition_broadcast(bvb[:], bv_s[:])

    # --- resident activations / inputs ------------------------------------
    xq_s = singles.tile([128, KI, S], FP16, tag="xq")
    xk_s = singles.tile([128, KI, S], FP16, tag="xk")
    xv_s = singles.tile([128, KI, S], FP16, tag="xv")
    # block 0 is split per contraction chunk so the first matmuls can
    # start ~1us in instead of waiting for a whole-block descriptor burst;
    # xk of each block is consumed last within emit_A, so defer it behind
    # the next block's xv/xq
    scol0 = ds(0, SB)
    for ic in range(KI):
        nc.sync.dma_start(xv_s[:, ic, scol0], xv[:, ic, scol0])
        nc.sync.dma_start(xq_s[:, ic, scol0], xq[:, ic, scol0])
    nc.sync.dma_start(xk_s[:, :, scol0], xk[:, :, scol0])
    for sb in range(1, N_SB):
        scol = ds(sb * SB, SB)
        nc.sync.dma_start(xv_s[:, :, scol], xv[:, :, scol])
        nc.sync.dma_start(xq_s[:, :, scol], xq[:, :, scol])
        nc.sync.dma_start(xk_s[:, :, scol], xk[:, :, scol])

    ident = singles.tile([128, 128], FP16, tag="ident")
    make_identity(nc, ident[:])
    # 4-fold tiled causal mask for masking a whole query block in one op
    umask4 = singles.tile([128, CPB, 128], FP16, tag="umask4")
    for cj in range(CPB):
        make_upper_triangular(nc, umask4[:, cj, :], val=1.0, diag=True)

    qpt = [singles.tile([128, S], FP16, tag=f"qpt{m}", name=f"qpt{m}") for m in range(2)]
    kpt = [singles.tile([128, S], FP16, tag=f"kpt{m}", name=f"kpt{m}") for m in range(2)]
    # pair-major vaug: vst[:, c, mt, 0:65] = [v | 1] of even head,
    # vst[:, c, mt, 65:130] = [v | 1] of odd head of pair mt
    vst = singles.tile([128, N_CH, 2, 130], FP16, tag="vst")
    nc.gpsimd.memset(vst[:, :, :, 64:65], 1.0)
    nc.gpsimd.memset(vst[:, :, :, 129:130], 1.0)
    outt = [singles.tile([128, S], FP16, tag=f"outt{c}", name=f"outt{c}") for c in range(2)]
    # pair-merged kv state, quadrant layout matching p_kv:
    # [rows 0:64, cols 0:65] = even head, [rows 64:128, cols 65:130] = odd
    kv2_sb = [
        singles.tile([128, 130], FP32, tag=f"kv2sb{mt}", name=f"kv2sb{mt}")
        for mt in range(2)
    ]
    for mt in range(2):
        nc.gpsimd.memset(kv2_sb[mt][:], 0.0)

    with (
        tc.tile_pool(name="phi", bufs=4) as phi_pool,
        tc.tile_pool(name="ssb", bufs=6) as ssb_pool,
        tc.tile_pool(name="knb", bufs=4) as kn_pool,
        tc.tile_pool(name="den", bufs=2) as den_pool,
        tc.tile_pool(name="kvs", bufs=4) as kvs_pool,
        tc.tile_pool(name="yt", bufs=3) as yt_pool,
    ):

        def phi_evict(p_x, b_x, m, dst, scol, pref):
            e_t = phi_pool.tile([128, SB], FP16, tag="e", name=f"e_{pref}")
            nc.scalar.activation(e_t[:], p_x[:], AF.Exp, bias=b_x[:, ds(m, 1)])
            r_t = phi_pool.tile([128, SB], FP16, tag="r", name=f"r_{pref}")
            nc.scalar.activation(r_t[:], p_x[:], AF.Relu, bias=b_x[:, ds(m, 1)])
            nc.vector.tensor_scalar_min(e_t[:], e_t[:], 1.0)
            nc.vector.tensor_add(dst[:, scol], r_t[:], e_t[:])

        def emit_A(sb, pqk_pool, pv_pool):
            scol = ds(sb * SB, SB)
            p_q = [pqk_pool.tile([128, SB], FP32, tag=f"m{m}", name=f"pq{m}_{sb}")
                   for m in range(2)]

            def qk_half(p_x, w_s, m):
                for ic in range(KI):
                    nc.tensor.matmul(
                        p_x[:], w_s[:, ic, ts(m, 128)], xq_s[:, ic, scol]
                        if w_s is wq_s else xk_s[:, ic, scol],
                        start=(ic == 0), stop=(ic == KI - 1),
                    )

            def v_half(hf):
                p_v = pv_pool.tile([128, SB], FP32, tag="v", name=f"pv{hf}_{sb}")
                for ic in range(KI):
                    for st in (2 * hf, 2 * hf + 1):
                        nc.tensor.matmul(
                            p_v[:, ds((st % 2) * O, O)],
                            xv_s[:, ic, ds(sb * SB + st * 128, 128)],
                            wv_s[:, ic, :],
                            start=(ic == 0 and st % 2 == 0),
                            stop=(ic == KI - 1 and st % 2 == 1),
                        )
                for st in (2 * hf, 2 * hf + 1):
                    c = sb * 4 + st
                    nc.vector.tensor_add(
                        vst[:, c].rearrange(
                            "p mt (two s) -> p mt two s", two=2)[:, :, :, 0:64],
                        p_v[:, ds((st % 2) * O, O)].rearrange(
                            "p (mt two d) -> p mt two d", mt=2, two=2),
                        bvb[:].rearrange("p (mt two d) -> p mt two d", mt=2, two=2),
                    )

            # order: q0, v01, q1, v23, k0, k1 — each PSUM reuse gap is
            # covered by >=1.7us of PE work while the evict chain drains
            qk_half(p_q[0], wq_s, 0)
            v_half(0)
            qk_half(p_q[1], wq_s, 1)
            v_half(1)
            for m in range(2):
                phi_evict(p_q[m], bq_s, m, qpt[m], scol, f"q{m}_{sb}")
            p_k = [pqk_pool.tile([128, SB], FP32, tag=f"m{m}", name=f"pk{m}_{sb}")
                   for m in range(2)]
            qk_half(p_k[0], wk_s, 0)
            qk_half(p_k[1], wk_s, 1)
            for m in range(2):
                phi_evict(p_k[m], bk_s, m, kpt[m], scol, f"k{m}_{sb}")

        def attn_pair(qb, pair, pnum_pool, pbig_pool, paux_pool):
            """Attention for a head pair (same mt) of query-block qb.
            PE matmuls batched by tile mode: [scores 64-mode],
            [num (128,128) mode], [kv stage], [inter x2 64-mode]."""
            upd_kv = qb < N_QB - 1
            mt = pair[0] // 2
            ctxs = []
            snaps = {}
            for h in pair:
                prow = 64 * (h % 2)
                p_num = pnum_pool.tile([65, QB], FP32, tag=f"n{h % 2}",
                                       name=f"num{qb}_{h}")
                if qb > 0:
                    snap = kvs_pool.tile([128, 65], FP16, tag="snap",
                                         name=f"snap{qb}_{h}")
                    nc.vector.tensor_copy(
                        snap[ds(prow, 64), :],
                        kv2_sb[mt][ds(prow, 64), ds(65 * (h % 2), 65)],
                    )
                    snaps[h] = snap
                ctxs.append((h, prow, qpt[mt][ds(prow, 64), :],
                             kpt[mt][ds(prow, 64), :], p_num))

            # -- scores, split diag / off-diagonal, all (64,128) mode -------
            # p_diag bank: the 4 causal diagonal chunks at cols cj*128;
            # p_off1 bank: cj=0 off (384 wide) at 0:384 + cj=2 off (128) at
            # 384:512; p_off2: cj=1 off (256 wide)
            sd_ts, so1_ts, so2_ts = {}, {}, {}
            for h, prow, qp_h, kp_h, p_num in ctxs:
                qb0 = qb * QB
                p_diag = pbig_pool.tile([128, QB], FP32, tag="big",
                                        name=f"pd{qb}_{h}")
                for cj in range(CPB):
                    c = qb * CPB + cj
                    nc.tensor.matmul(
                        p_diag[:, ts(cj, CH)], kp_h[:, ds(c * CH, CH)],
                        qp_h[:, ds(qb0 + cj * CH, CH)],
                        start=(cj == 0), stop=(cj == CPB - 1),
                    )
                p_off1 = pbig_pool.tile([128, QB], FP32, tag="big",
                                        name=f"po1{qb}_{h}")
                nc.tensor.matmul(
                    p_off1[:, 0:384], kp_h[:, ds((qb * CPB) * CH, CH)],
                    qp_h[:, ds(qb0 + CH, 384)], start=True, stop=False,
                )
                nc.tensor.matmul(
                    p_off1[:, 384:512], kp_h[:, ds((qb * CPB + 2) * CH, CH)],
                    qp_h[:, ds(qb0 + 384, 128)], start=False, stop=True,
                )
                p_off2 = pbig_pool.tile([128, 256], FP32, tag="big",
                                        name=f"po2{qb}_{h}")
                nc.tensor.matmul(
                    p_off2[:], kp_h[:, ds((qb * CPB + 1) * CH, CH)],
                    qp_h[:, ds(qb0 + 2 * CH, 256)], start=True, stop=True,
                )
                # evictions: one masked mul + two plain copies per head
                s_d = ssb_pool.tile([128, QB], FP16, tag="sd", name=f"sd{qb}_{h}")
                nc.vector.tensor_mul(s_d[:], p_diag[:], umask4[:])
                s_o1 = ssb_pool.tile([128, QB], FP16, tag="so1",
                                     name=f"so1{qb}_{h}")
                nc.scalar.copy(s_o1[:], p_off1[:])
                s_o2 = ssb_pool.tile([128, 256], FP16, tag="so2",
                                     name=f"so2{qb}_{h}")
                nc.scalar.copy(s_o2[:], p_off2[:])
                sd_ts[h], so1_ts[h], so2_ts[h] = s_d, s_o1, s_o2
            # -- num: 7 matmuls per head, (128,128) mode --------------------
            for h, prow, qp_h, kp_h, p_num in ctxs:
                def vslc(cj):
                    return vst[:, qb * CPB + cj, mt, ds(65 * (h % 2), 65)]
                nc.tensor.matmul(p_num[:, ts(0, CH)], vslc(0),
                                 sd_ts[h][:, ts(0, CH)], start=True, stop=False)
                nc.tensor.matmul(p_num[:, ds(CH, 384)], vslc(0),
                                 so1_ts[h][:, 0:384], start=False, stop=False)
                nc.tensor.matmul(p_num[:, ts(1, CH)], vslc(1),
                                 sd_ts[h][:, ts(1, CH)], start=False, stop=False)
                nc.tensor.matmul(p_num[:, ds(2 * CH, 256)], vslc(1),
                                 so2_ts[h][:], start=False, stop=False)
                nc.tensor.matmul(p_num[:, ts(2, CH)], vslc(2),
                                 sd_ts[h][:, ts(2, CH)], start=False, stop=False)
                nc.tensor.matmul(p_num[:, ds(3 * CH, CH)], vslc(2),
                                 so1_ts[h][:, 384:512], start=False, stop=False)
                nc.tensor.matmul(p_num[:, ts(3, CH)], vslc(3),
                                 sd_ts[h][:, ts(3, CH)], start=False,
                                 stop=(qb == 0))
            if upd_kv:
                # -- pair transposes: 4 matmuls into one PSUM bank ----------
                p_t4 = pbig_pool.tile([128, CPB, 128], FP16, tag="big",
                                      name=f"pt4_{qb}_{mt}")
                for cj in range(CPB):
                    c = qb * CPB + cj
                    nc.tensor.matmul(
                        p_t4[:, cj, :], kpt[mt][:, ds(c * CH, CH)], ident[:],
                        is_transpose=True,
                        start=(cj == 0), stop=(cj == CPB - 1),
                    )
                # single merged eviction of all 4 transposed chunks
                kn4 = kn_pool.tile([128, CPB, 128], FP16, tag="kn",
                                   name=f"kn4_{qb}_{mt}")
                nc.vector.tensor_copy(kn4[:], p_t4[:])
                # -- pair kv updates: 4 matmuls accumulated in PSUM ---------
                # out rows 0-63 = even head feats, 64-127 = odd head feats;
                # cols 0-64 = even head vaug, 65-129 = odd head vaug.
                # Only the two matching quadrants are used.
                p_kv = pbig_pool.tile([128, 130], FP32, tag="big",
                                      name=f"pkv{qb}_{mt}")
                for cj in range(CPB):
                    c = qb * CPB + cj
                    nc.tensor.matmul(
                        p_kv[:], kn4[:, cj, :], vst[:, c, mt, :],
                        start=(cj == 0), stop=(cj == CPB - 1),
                    )
                # single pair-state add; the two mismatched quadrants carry
                # cross-head garbage that is never read
                nc.vector.tensor_add(kv2_sb[mt][:], kv2_sb[mt][:], p_kv[:])
            # -- inter: 2 matmuls, (64,128) mode ----------------------------
            if qb > 0:
                for h, prow, qp_h, kp_h, p_num in ctxs:
                    nc.tensor.matmul(
                        p_num[:], snaps[h][ds(prow, 64), :],
                        qp_h[:, ds(qb * QB, QB)], start=False, stop=True,
                    )
            # -- den / outt (pair-merged reciprocal chain, free-dim packed) --
            den2 = den_pool.tile([1, 2, QB], FP32, tag="den", name=f"den{qb}_{mt}")
            for h, prow, qp_h, kp_h, p_num in ctxs:
                nc.scalar.copy(den2[:, h % 2, :], p_num[ds(64, 1), :])
            rden2 = den_pool.tile([1, 2, QB], FP32, tag="rden", name=f"rden{qb}_{mt}")
            nc.vector.reciprocal_approx_fast(rden2[:], den2[:])
            rd16 = den_pool.tile([1, 2, QB], FP16, tag="rd16", name=f"rd16{qb}_{mt}")
            nc.vector.tensor_copy(rd16[:], rden2[:])
            for h, prow, qp_h, kp_h, p_num in ctxs:
                bc_t = den_pool.tile([64, QB], FP16, tag="bc", name=f"bc{qb}_{h}")
                nc.gpsimd.partition_broadcast(bc_t[:], rd16[:, h % 2, :])
                nc.vector.tensor_mul(
                    outt[mt][ds(prow, 64), ds(qb * QB, QB)], p_num[0:64, :], bc_t[:]
                )

        def emit_C(qb, pbig_pool):
            for st in range(qb * CPB, (qb + 1) * CPB):
                y_t = yt_pool.tile([128, H_DIM], FP16, tag="y", name=f"yt{st}")
                for n in range(2):
                    p_o = pbig_pool.tile([128, 512], FP32, tag="big",
                                         name=f"po{st}_{n}")
                    for ct in range(2):
                        nc.tensor.matmul(
                            p_o[:], outt[ct][:, ts(st, 128)],
                            wo_s[:, ct, ts(n, 512)],
                            start=(ct == 0), stop=(ct == 1),
                        )
                    if n == 0:
                        nc.scalar.copy(y_t[:, ts(n, 512)], p_o[:])
                    else:
                        nc.vector.tensor_copy(y_t[:, ts(n, 512)], p_o[:])
                nc.gpsimd.dma_start(y[ds(st * 128, 128), :], y_t[:])

        # single PSUM allocation for the whole kernel — no mid-kernel
        # pool-transition barrier: 2 (qk) + 1 (v) + 2 (num) + 3 (flow) = 8
        with (
            tc.tile_pool(name="pqk", bufs=1, space="PSUM") as pqk_pool,
            tc.tile_pool(name="pv", bufs=1, space="PSUM") as pv_pool,
            tc.tile_pool(name="pnum", bufs=1, space="PSUM") as pnum_e,
            tc.tile_pool(name="pbig", bufs=3, space="PSUM") as pbig_e,
        ):
            emit_A(0, pqk_pool, pv_pool)
            emit_A(1, pqk_pool, pv_pool)
            attn_pair(0, [0, 1], pnum_e, pbig_e, None)
            attn_pair(0, [2, 3], pnum_e, pbig_e, None)
            emit_A(2, pqk_pool, pv_pool)
            emit_C(0, pbig_e)
            attn_pair(1, [0, 1], pnum_e, pbig_e, None)
            attn_pair(1, [2, 3], pnum_e, pbig_e, None)
            emit_A(3, pqk_pool, pv_pool)
            emit_C(1, pbig_e)
            attn_pair(2, [0, 1], pnum_e, pbig_e, None)
            attn_pair(2, [2, 3], pnum_e, pbig_e, None)
            attn_pair(3, [0, 1], pnum_e, pbig_e, None)
            emit_C(2, pbig_e)
            attn_pair(3, [2, 3], pnum_e, pbig_e, None)
            emit_C(3, pbig_e)


_PROGRAM = None


def _get_program():
    global _PROGRAM
    if _PROGRAM is None:
        nc = bacc.Bacc("TRN2", target_bir_lowering=False, debug=False)
        with tile.TileContext(nc) as tc:
            _emit(tc)
        nc.compile()
        _PROGRAM = nc
    return _PROGRAM


def kernel(query, key, value, Wq, bq, Wk, bk, Wv, bv, Wo, bo, _trace=False):
    query, key, value = (np.asarray(a, np.float32) for a in (query, key, value))
    Wq, Wk, Wv, Wo = (np.asarray(a, np.float32) for a in (Wq, Wk, Wv, Wo))
    bq, bk, bv, bo = (np.asarray(a, np.float32) for a in (bq, bk, bv, bo))

    def xslice(x):  # (2048, 1024) -> (128, 8, 2048) fp16, contraction-chunked
        return np.ascontiguousarray(
            x.T.reshape(KI, 128, S).transpose(1, 0, 2)
        ).astype(np.float16)

    def wslice(W, g):  # (1024, 256) -> (128, 8, 256) contraction-chunked
        wt = W[g * O:(g + 1) * O].T  # (1024, 256)
        return np.ascontiguousarray(
            wt.reshape(KI, 128, O).transpose(1, 0, 2)
        ).astype(np.float16)

    xmaps = [
        {
            "xq": xslice(query[b]),
            "xk": xslice(key[b]),
            "xv": xslice(value[b]),
        }
        for b in range(B)
    ]
    in_maps = []
    for c in range(N_CORES):
        b, g = divmod(c, 4)
        sl = slice(g * O, (g + 1) * O)
        in_maps.append({
            **xmaps[b],
            "wq": wslice(Wq, g),
            "wk": wslice(Wk, g),
            "wv": wslice(Wv, g),
            "wo": np.ascontiguousarray(
                Wo[:, sl].T.reshape(2, 128, H_DIM).transpose(1, 0, 2)
            ).astype(np.float16),
            "bq": np.ascontiguousarray(bq[sl].reshape(2, 128).T),
            "bk": np.ascontiguousarray(bk[sl].reshape(2, 128).T),
            "bv": np.ascontiguousarray(bv[sl].reshape(1, O)).astype(np.float16),
        })

    nc = _get_program()
    res = run_bass_kernel_spmd(
        nc, in_maps, core_ids=list(range(N_CORES)), trace=_trace
    )
    out = np.empty((B, S, H_DIM), np.float32)
    for b in range(B):
        acc = res.results[4 * b]["y"].astype(np.float64)
        for g in range(1, 4):
            acc += res.results[4 * b + g]["y"]
        out[b] = (acc + bo).astype(np.float32)
    if _trace:
        kernel.last_result = res
    return out
